# revision 44
# baseline (speedup 1.0000x reference)
"""Trainium2 Bass kernel v3 for GroundwaterModel Jacobi pseudo-timestepping.

Layout (per core, x sharded 128 rows/core):
  partition p (0..127), block b (0..7)  <->  y = 8p + b   (y interleaved)
  in-block free col m (0..F-1)          <->  x = 128c - (H+1) + m
  F = 130 + 2H:  [pad | H left ghosts | 128 owned | H right ghosts | pad]
  One SBUF tile q[128, F*8] fp16 holds the full per-core state.

Update p' = A.p_xp + B.p_xm + C.p_yp + D.p_ym + E:
  - DVE computes all 4 coefficient streams in one broadcast mul per chunk:
    s[:, k, r] = cc[:, k, r] * q[:, r]  (cc = [dsh,csh,ash,bsh] stacked);
    the wrap-region chunks (symw, C) run on the Pool engine (SBUF only --
    GPSIMD cannot touch PSUM).
  - PE accumulates E + the 4 shifted streams via identity / partition-shift
    matmuls (shifts = read offsets; y wrap via supT/sdnT) into SIX per-region
    PSUM tiles (each padded to its own physical bank), so each ACT drain
    depends only on its own region's accumulation chain.
  - ACT drains each region PSUM -> q fp16 as soon as its chain stops.
  Chunk/region boundaries [0,348,512,676,840,1024,1148,1312) are co-designed
  with the y+-F couplings so drains retire in chunk order and unblock the
  next step's mul chunks with minimal stall (sim ~3.1us/step steady state).

Halo: ghosts H=17 wide refresh every 17 steps (5 refreshes) via two
AllGathers-free scheme: one AllGather of own margins and data-driven mask
selection; grid-edge cores use mirrored ghosts (reflection principle ==
reference's edge replication).  Measured ~free on HW.
"""

import numpy as np

GRID = 1024
NCORES = 8
P = 128
H = 17
F = 130 + 2 * H          # 164
FF = 8 * F               # 1312
NB = 8
OWN0 = H + 1             # first owned in-block col (18)
TS = 100

_cached = {}
DISABLE_REFRESH = False


def _host_inputs(u, f, n_cores, time_steps):
    N = u.shape[0]
    h = 1.0 / (N - 1)
    u = u.astype(np.float64)
    f = f.astype(np.float64)
    eu = np.exp(u)
    eu_xm = np.concatenate([eu[:1, :], eu[:-1, :]], 0)
    eu_ym = np.concatenate([eu[:, :1], eu[:, :-1]], 1)
    den = 2.0 * eu + eu_xm + eu_ym
    A = eu / den
    B = eu_xm / den
    C = eu / den
    D = eu_ym / den
    E = (h * h) * f / den
    for arr in (A, B, C, D):
        arr[:, 0] = 0.0
        arr[:, -1] = 0.0
    xs = np.arange(N, dtype=np.float64) * h
    E[:, 0] = xs
    E[:, -1] = 1.0 - xs

    def til(arr):  # [F, N] fp -> [128, 8F] fp16 tile layout
        return np.ascontiguousarray(
            arr.reshape(F, P, NB).transpose(1, 2, 0).reshape(P, NB * F)
        ).astype(np.float16)

    in_maps = []
    for c in range(n_cores):
        r0 = c * P
        At = np.zeros((F, N)); Bt = np.zeros((F, N))
        Ct = np.zeros((F, N)); Dt = np.zeros((F, N)); Et = np.zeros((F, N))
        for m in range(1, F - 1):
            x = r0 - (H + 1) + m
            if 0 <= x < N:
                At[m], Bt[m], Ct[m], Dt[m], Et[m] = A[x], B[x], C[x], D[x], E[x]
            else:
                xt = -1 - x if x < 0 else 2 * N - 1 - x
                # mirror: x-direction roles swap
                At[m], Bt[m], Ct[m], Dt[m], Et[m] = B[xt], A[xt], C[xt], D[xt], E[xt]
        # source-aligned (storage-space) shifts
        Ash = np.zeros_like(At); Ash[1:] = At[:-1]
        Bsh = np.zeros_like(Bt); Bsh[:-1] = Bt[1:]
        Csh = np.zeros_like(Ct); Csh[:, 1:] = Ct[:, :-1]
        Dsh = np.zeros_like(Dt); Dsh[:, :-1] = Dt[:, 1:]

        # stream order k: 0=sym(dsh), 1=syp(csh), 2=sxp(ash), 3=sxm(bsh)
        cc = np.stack([til(Dsh), til(Csh), til(Ash), til(Bsh)], axis=1)

        ident16 = np.eye(P, dtype=np.float16)
        supT = np.zeros((P, P), dtype=np.float16)
        for p in range(P - 1):
            supT[p + 1, p] = 1.0    # out[p] = in[p+1]
        sdnT = np.zeros((P, P), dtype=np.float16)
        for p in range(1, P):
            sdnT[p - 1, p] = 1.0    # out[p] = in[p-1]
        ident32 = np.eye(P, dtype=np.float32)

        def mk_mask(val):
            return np.full((P, NB, H), val, dtype=np.uint8)

        # world-AG selection masks: left ghosts come from slot c-1's right
        # section (mirror of own for core 0); right from slot c+1's left
        # section (mirror for core 7).
        im = {
            "cc": np.ascontiguousarray(cc), "e0": til(Et),
            "ident16": ident16, "supT": supT, "sdnT": sdnT, "ident32": ident32,
        }
        for s in range(1, n_cores - 1):
            im[f"mL{s}"] = mk_mask(1.0 if c == s + 1 else 0.0)
            im[f"mR{s}"] = mk_mask(1.0 if c == s - 1 else 0.0)
        im["mR7"] = mk_mask(1.0 if c == n_cores - 2 else 0.0)
        im["mLM"] = mk_mask(1.0 if c == 0 else 0.0)
        im["mRM"] = mk_mask(1.0 if c == n_cores - 1 else 0.0)
        in_maps.append(im)
    return in_maps


def _build(n_cores, time_steps, repeats=1, single_core_profile=False):
    import concourse.bass as bass
    import concourse.bacc as bacc
    import concourse.mybir as mybir
    from concourse.tile import TileContext

    f16 = mybir.dt.float16
    f32 = mybir.dt.float32
    AF = mybir.ActivationFunctionType
    OP = mybir.AluOpType

    nc = bacc.Bacc("TRN2", target_bir_lowering=False, debug=False,
                   num_devices=(1 if single_core_profile else n_cores))
    dp = nc.declare_dram_parameter
    cc_d = dp("cc", [P, 4, FF], f16, isOutput=False)
    e0_d = dp("e0", [P, FF], f16, isOutput=False)
    i16_d = dp("ident16", [P, P], f16, isOutput=False)
    sup_d = dp("supT", [P, P], f16, isOutput=False)
    sdn_d = dp("sdnT", [P, P], f16, isOutput=False)
    i32_d = dp("ident32", [P, P], f32, isOutput=False)
    mask_names = ([f"mL{s}" for s in range(1, n_cores - 1)]
                  + [f"mR{s}" for s in range(1, n_cores)]
                  + ["mLM", "mRM"])
    u8 = mybir.dt.uint8
    mask_d = {nm: dp(nm, [P, NB, H], u8, isOutput=False)
              for nm in mask_names}
    pout_d = dp("pout", [P, NB, P], f32, isOutput=True)

    rg = [list(range(n_cores))]

    refresh_steps = set()
    t = 1 + H
    while t < time_steps:
        refresh_steps.add(t)
        t += H
    if single_core_profile or DISABLE_REFRESH:
        refresh_steps = set()

    F7 = 7 * F

    with TileContext(nc) as tc:
        with (
            tc.tile_pool(name="coef", bufs=1) as coef,
            tc.tile_pool(name="work", bufs=2) as work,
            tc.tile_pool(name="qp", bufs=2, space="PSUM") as qp,
            tc.tile_pool(name="dramp", bufs=2, space="DRAM") as dramp,
        ):
            cc = coef.tile([P, 4, FF], f16, name="cc_t")
            e0 = coef.tile([P, FF], f16, name="e0_t")
            i16 = coef.tile([P, P], f16, name="i16_t")
            supT = coef.tile([P, P], f16, name="sup_t")
            sdnT = coef.tile([P, P], f16, name="sdn_t")
            i32 = coef.tile([P, P], f32, name="i32_t")
            masks = {nm: coef.tile([P, NB, H], mybir.dt.uint8,
                                   name=f"{nm}_t")
                     for nm in mask_d}
            outsb = coef.tile([P, NB * P], f32, name="outsb")
            nc.sync.dma_start(out=cc[:, :, :], in_=cc_d[:, :, :])
            for sb_t, d_t in [(e0, e0_d), (i16, i16_d),
                              (supT, sup_d), (sdnT, sdn_d), (i32, i32_d)]:
                nc.sync.dma_start(out=sb_t[:, :], in_=d_t[:, :])
            for nm in mask_d:
                nc.sync.dma_start(out=masks[nm][:, :, :], in_=mask_d[nm][:, :, :])

            V = nc.vector
            G = nc.gpsimd
            mm = nc.tensor.matmul

            def mul_chunk(s, q, lo, hi, k0=0, k1=4, eng=None):
                nk = k1 - k0
                qb = q[:, lo:hi].rearrange("p (k m) -> p k m", k=1)
                qb = qb.broadcast_to([P, nk, hi - lo])
                (eng or V).tensor_mul(s[:, k0:k1, lo:hi], cc[:, k0:k1, lo:hi],
                                      qb)

            def refresh(q, t):
                Qr = q[:, :].rearrange("p (b f) -> p b f", b=NB)
                contrib = work.tile([P, 2, NB, H], f16, tag="contrib",
                                    name=f"contrib_{t}")
                V.tensor_copy(contrib[:, 0, :, :], Qr[:, :, OWN0:OWN0 + H])
                V.tensor_copy(contrib[:, 1, :, :],
                              Qr[:, :, OWN0 + P - H:OWN0 + P])
                bounce = dramp.tile([P, 2, NB, H], f16, tag="bounce",
                                    name=f"bounce_{t}")
                nc.sync.dma_start(out=bounce[:, :, :, :],
                                  in_=contrib[:, :, :, :])
                gath = dramp.tile([n_cores * P, 2, NB, H], f16, tag="gath",
                                  addr_space="Shared", name=f"gath_{t}")
                nc.gpsimd.collective_compute(
                    "AllGather", mybir.AluOpType.bypass,
                    ins=[bounce.opt()], outs=[gath.opt()],
                    replica_groups=rg)
                gsb = work.tile([P, n_cores, 2, NB, H], f16, tag="gsb",
                                name=f"gsb_{t}")
                for s in range(n_cores):
                    nc.sync.dma_start(out=gsb[:, s, :, :, :],
                                      in_=gath[s * P:(s + 1) * P, :, :, :])
                # left ghosts <- slot c-1 right-section; core 0 mirrors own
                gl = work.tile([P, NB, H], f16, tag="gl", name=f"gl_{t}")
                V.tensor_copy(gl[:, :, :], gsb[:, 0, 1, :, :])
                for s in range(1, n_cores - 1):
                    V.copy_predicated(gl[:, :, :], masks[f"mL{s}"][:, :, :],
                                      gsb[:, s, 1, :, :])
                V.copy_predicated(gl[:, :, :], masks["mLM"][:, :, :],
                                  Qr[:, :, OWN0 + H - 1:OWN0 - 1:-1])
                V.tensor_copy(Qr[:, :, 1:1 + H], gl[:, :, :])
                # right ghosts <- slot c+1 left-section; core 7 mirrors own
                gr = work.tile([P, NB, H], f16, tag="gr", name=f"gr_{t}")
                V.tensor_copy(gr[:, :, :], gsb[:, 1, 0, :, :])
                for s in range(2, n_cores):
                    V.copy_predicated(gr[:, :, :], masks[f"mR{s}"][:, :, :],
                                      gsb[:, s, 0, :, :])
                V.copy_predicated(gr[:, :, :], masks["mRM"][:, :, :],
                                  Qr[:, :, OWN0 + P - 1:OWN0 + P - H - 1:-1])
                V.tensor_copy(Qr[:, :, OWN0 + P:OWN0 + P + H], gr[:, :, :])

            for rep in range(repeats):
                q = work.tile([P, FF], f16, tag="q", name=f"q1_r{rep}")
                V.tensor_copy(q[:, :], e0[:, :])
                pf = None
                for t in range(2, time_steps + 1):
                    s = work.tile([P, 4, FF], f16, tag="s",
                                  name=f"s_{rep}_{t}")
                    # DVE chunk order; each chunk's q-range is unblocked by
                    # one drain of the previous step:
                    #   A    <- drain_A   symw/W/C <- drain_C   B1/B2 <- drain_B
                    mul_chunk(s, q, 0, 348)                  # A1     (DVE)
                    mul_chunk(s, q, F7, FF, 0, 1, eng=G)     # symw   (Pool)
                    mul_chunk(s, q, 1024, F7, eng=G)         # C      (Pool)
                    mul_chunk(s, q, 348, 512)                # A2     (DVE)
                    mul_chunk(s, q, 512, 676)                # B1     (DVE)
                    mul_chunk(s, q, 676, 840)                # B2a    (DVE)
                    mul_chunk(s, q, 840, 1024)               # B2b    (DVE)
                    mul_chunk(s, q, F7, FF, 1, 4)            # W      (DVE, last)
                    # one PSUM tile per 124..348-col region (PSUM slots pad
                    # to full banks): each drain depends only on its own
                    # region's accumulation chain, so drains retire in
                    # chunk order and unblock the next step's muls early.
                    psA1 = qp.tile([P, 348], f32, tag="psA1", bufs=1,
                                   name=f"psA1_{rep}_{t}")
                    psA2 = qp.tile([P, 164], f32, tag="psA2", bufs=1,
                                   name=f"psA2_{rep}_{t}")
                    psB1 = qp.tile([P, 164], f32, tag="psB1", bufs=1,
                                   name=f"psB1_{rep}_{t}")
                    psB2 = qp.tile([P, 348], f32, tag="psB2", bufs=1,
                                   name=f"psB2_{rep}_{t}")
                    psC1 = qp.tile([P, 124], f32, tag="psC1", bufs=1,
                                   name=f"psC1_{rep}_{t}")
                    psC2 = qp.tile([P, 164], f32, tag="psC2", bufs=1,
                                   name=f"psC2_{rep}_{t}")
                    sym = s[:, 0, :]; syp = s[:, 1, :]
                    sxp = s[:, 2, :]; sxm = s[:, 3, :]
                    # drain targets
                    if t < time_steps:
                        qn = work.tile([P, FF], f16, tag="q",
                                       name=f"q_{rep}_{t}")
                    else:
                        qn = work.tile([P, FF], f32, tag="pf",
                                       name=f"pf_{rep}")
                        pf = qn
                    ACT = nc.scalar.activation
                    # E inits (dep-free bank clears)
                    mm(psA1[:, 0:348], i16[:, :], e0[:, 0:348],
                       start=True, stop=False)
                    mm(psA2[:, 0:164], i16[:, :], e0[:, 348:512],
                       start=True, stop=False)
                    mm(psB1[:, 0:164], i16[:, :], e0[:, 512:676],
                       start=True, stop=False)
                    mm(psB2[:, 0:348], i16[:, :], e0[:, 676:1024],
                       start=True, stop=False)
                    mm(psC1[:, 0:124], i16[:, :], e0[:, 1024:F7],
                       start=True, stop=False)
                    mm(psC2[:, 0:164], i16[:, :], e0[:, F7:FF],
                       start=True, stop=False)
                    # --- fed by A1 ---
                    mm(psA1[:, F:348], i16[:, :], sym[:, 0:348 - F],
                       start=False, stop=False)                     # ym_A1
                    mm(psA1[:, 1:348], i16[:, :], sxm[:, 0:347],
                       start=False, stop=False)                     # xm_A1
                    mm(psA1[:, 0:184], i16[:, :], syp[:, F:348],
                       start=False, stop=False)                     # yp_A1a
                    mm(psA1[:, 0:347], i16[:, :], sxp[:, 1:348],
                       start=False, stop=False)                     # xp_A1a
                    mm(psA2[:, 0:164], i16[:, :], sym[:, 348 - F:348],
                       start=False, stop=False)                     # ym_A2
                    mm(psC2[:, 0:164], supT[:, :], syp[:, 0:F],
                       start=False, stop=False)                     # wrap2
                    # --- fed by symw (Pool) ---
                    mm(psA1[:, 0:F], sdnT[:, :], sym[:, F7:FF],
                       start=False, stop=False)                     # wrap1
                    # --- fed by A2 ---
                    mm(psA1[:, 347:348], i16[:, :], sxp[:, 348:349],
                       start=False, stop=False)                     # xp_A1b
                    mm(psA1[:, 184:348], i16[:, :], syp[:, 348:512],
                       start=False, stop=True)                      # yp_A1b
                    ACT(qn[:, 0:348], psA1[:, 0:348], AF.Copy)      # drain_A1
                    mm(psA2[:, 0:164], i16[:, :], sxm[:, 347:511],
                       start=False, stop=False)                     # xm_A2
                    mm(psA2[:, 0:163], i16[:, :], sxp[:, 349:512],
                       start=False, stop=False)                     # xp_A2a
                    mm(psB1[:, 0:164], i16[:, :], sym[:, 348:512],
                       start=False, stop=False)                     # ym_B1
                    # --- fed by B1 ---
                    mm(psA2[:, 163:164], i16[:, :], sxp[:, 512:513],
                       start=False, stop=False)                     # xp_A2b
                    mm(psA2[:, 0:164], i16[:, :], syp[:, 512:676],
                       start=False, stop=True)                      # yp_A2
                    ACT(qn[:, 348:512], psA2[:, 0:164], AF.Copy)    # drain_A2
                    mm(psB1[:, 0:164], i16[:, :], sxm[:, 511:675],
                       start=False, stop=False)                     # xm_B1
                    mm(psB1[:, 0:163], i16[:, :], sxp[:, 513:676],
                       start=False, stop=False)                     # xp_B1a
                    mm(psB2[:, 0:164], i16[:, :], sym[:, 512:676],
                       start=False, stop=False)                     # ym_B2a
                    # --- fed by B2a ---
                    mm(psB1[:, 163:164], i16[:, :], sxp[:, 676:677],
                       start=False, stop=False)                     # xp_B1b
                    mm(psB1[:, 0:164], i16[:, :], syp[:, 676:840],
                       start=False, stop=True)                      # yp_B1
                    ACT(qn[:, 512:676], psB1[:, 0:164], AF.Copy)    # drain_B1
                    # --- fed by B2b ---
                    mm(psB2[:, 164:348], i16[:, :], sym[:, 676:860],
                       start=False, stop=False)                     # ym_B2b
                    mm(psB2[:, 0:348], i16[:, :], sxm[:, 675:1023],
                       start=False, stop=False)                     # xm_B2
                    mm(psB2[:, 0:347], i16[:, :], sxp[:, 677:1024],
                       start=False, stop=False)                     # xp_B2a
                    mm(psB2[:, 0:164], i16[:, :], syp[:, 840:1004],
                       start=False, stop=False)                     # yp_B2a
                    mm(psC1[:, 0:124], i16[:, :], sym[:, 860:984],
                       start=False, stop=False)                     # ym_C1
                    # --- fed by C (Pool) ---
                    mm(psB2[:, 347:348], i16[:, :], sxp[:, 1024:1025],
                       start=False, stop=False)                     # xp_B2b
                    mm(psC2[:, 0:164], i16[:, :], sym[:, 984:1148],
                       start=False, stop=False)                     # ym_C2
                    mm(psC1[:, 0:124], i16[:, :], sxm[:, 1023:1147],
                       start=False, stop=False)                     # xm_C1
                    # --- fed by W ---
                    # C2 chain first: drain_C2 gates symw(t+1) -> wrap1(t+1)
                    # -> stop_A1(t+1); drain_B2's consumer (B2a mul) has
                    # far more slack.
                    mm(psC2[:, 0:164], i16[:, :], sxm[:, 1147:1311],
                       start=False, stop=False)                     # xm_C2
                    mm(psC2[:, 0:163], i16[:, :], sxp[:, 1149:FF],
                       start=False, stop=True)                      # xp_C2
                    ACT(qn[:, F7:FF], psC2[:, 0:164], AF.Copy)      # drain_C2
                    mm(psB2[:, 164:348], i16[:, :], syp[:, 1004:1188],
                       start=False, stop=True)                      # yp_B2b
                    ACT(qn[:, 676:1024], psB2[:, 0:348], AF.Copy)   # drain_B2
                    mm(psC1[:, 0:124], i16[:, :], sxp[:, 1025:1149],
                       start=False, stop=False)                     # xp_C1
                    mm(psC1[:, 0:124], i16[:, :], syp[:, 1024 + F:FF],
                       start=False, stop=True)                      # yp_C1
                    ACT(qn[:, 1024:F7], psC1[:, 0:124], AF.Copy)    # drain_C1
                    if t < time_steps:
                        q = qn
                        if t in refresh_steps:
                            refresh(q, f"{rep}_{t}")

            # final transpose to [x, y-blockmajor] and write out
            for b in range(NB):
                # full-bank tile: transpose's start=True clears the whole
                # physical bank; reuse the psB ring (loop is done with it).
                pst = qp.tile([P, P], f32, tag="pst", bufs=2,
                              name=f"pst_{b}")
                nc.tensor.transpose(pst[:, 0:P],
                                    pf[:, b * F + OWN0:b * F + OWN0 + P],
                                    i32[:, :])
                nc.scalar.activation(outsb[:, b * P:(b + 1) * P],
                                     pst[:, 0:P], AF.Copy)
            for b in range(NB):
                nc.sync.dma_start(out=pout_d[:, b, :],
                                  in_=outsb[:, b * P:(b + 1) * P])

    nc.finalize()
    return nc


def _get_nc(n_cores, time_steps, repeats=1):
    key = (n_cores, time_steps, repeats)
    if key not in _cached:
        _cached[key] = _build(n_cores, time_steps, repeats)
    return _cached[key]


def kernel(u, f, time_steps):
    from concourse.bass_utils import run_bass_kernel_spmd

    u = np.asarray(u)
    f = np.asarray(f)
    ts = int(time_steps)
    N = u.shape[0]
    nc = _get_nc(NCORES, ts)
    in_maps = _host_inputs(u, f, NCORES, ts)
    res = run_bass_kernel_spmd(nc, in_maps, list(range(NCORES))).results
    out = np.empty((N, N), dtype=np.float32)
    for c in range(NCORES):
        po = res[c]["pout"]  # [128, 8, 128] -> [128 x, 1024 y]
        out[c * P:(c + 1) * P] = po.transpose(0, 2, 1).reshape(P, N)
    return out
